# revision 34
# baseline (speedup 1.0000x reference)
"""Trainium2 Bass kernel for the ContrastiveModel loss.

Math (per batch b):
    z1 = proj(X1[b]), z2 = proj(X2[b]);  proj(x) = elu(x@W1.T+b1)@W2.T+b2
    z1n, z2n = L2-normalized rows
    E11 = exp(z1n z1n^T / tau), E12 = exp(z1n z2n^T / tau), E22 likewise
    l1 = sum_l [log(rowsum(E11)+rowsum(E12)-e^2) - 2*s12[l]]
    l2 = sum_l [log(rowsum(E22)+colsum(E12)-e^2) - 2*s12[l]]
    loss = mean_b 0.5*(l1+l2),  s12[l] = z1n[l].z2n[l]

Sharding: 8 cores, 2 per batch; each core owns a 2048-row block of the sim
matrices in a rolled coordinate frame (host rolls the L axis by the shard
offset so one SPMD program serves all cores; rolled rows 0..2048 are the
core's own rows, rolled cols 2048..4096 the partner's).

E11/E22 are symmetric: each unordered tile pair is computed once.  Per
128-row tile t the core computes region A = cols [t*128, 2048) and region
B = cols [2048+t*128, 4096).  The leading 128-col tile of each region (the
diagonal / anti-diagonal tile) contributes row sums only; every later tile
contributes its row sums via the activation accum and its transposed row
sums via a column accumulator.  Host combine: den rows of core s =
accum_s + colacc_s[0:2048] + colacc_partner[2048:4096].  The E22 and E12
column accumulators share one buffer (both feed den2); E11 has its own
(den1).  E12 is not symmetric and is computed in full.

Pipeline per core (software-pipelined for ScalarE occupancy -- on HW the
activation engine runs faster than the cost model, so hiding E11 under the
second projection measures faster than the phase-sequential layout):
  1. proj1 with norm1-A interleaved per chunk (squares on Pool, ones-matmul
     column sums staged to SBUF -- no ScalarE work), then norm1-B: one
     batched Ln over [1,L] (avoids activation-table thrash) and
     rnorm = exp(-ln(|z|^2)/2) fused into the broadcast PSUM->SBUF copy;
     zb1 = zt1 * rnorm in fp8e4m3.  elu(v)+1 = min(exp(v), relu(v)+1) with
     the -1 folded into b2' = b2 - W2.sum(1) on the host (bf16-rounded W2
     to match the device matmul).
  2. proj2 with E11 sim tiles (grain 1024) AND norm2-A interleaved per
     chunk; then norm2-B (+ on-chip s12 scalar) while E11 drains; cacc11's
     column sums stream out right after.
  3. E12 + E22 sims: fp8 DoubleRow matmuls (K=256 per instruction, 0.5
     cyc/row) fill [128,2048] PSUM blocks; ScalarE exp(2x) with fused
     row-sum (accum_out); column sums accumulate on DVE (bf16 2x).
  4. one packed output: [rA | rB | cs11 | csX | s12].
"""

import numpy as np

import concourse.bass as bass
import concourse.mybir as mybir
import concourse.tile as tile
from concourse import bacc

F32 = mybir.dt.float32
BF16 = mybir.dt.bfloat16
F8 = mybir.dt.float8e4
AF = mybir.ActivationFunctionType
ALU = mybir.AluOpType
DR = mybir.MatmulPerfMode.DoubleRow

B, L, D = 4, 4096, 256
NCORES = 8
SHARD = L // 2            # rows of the sim matrices per core
NT = SHARD // 128         # 16 l-tiles per core
NMC = L // 512            # 8 chunks of 512
XCHUNK = 1024             # x DMA burst width
GRAIN = 2048              # E12 column-group width (4 PSUM banks)
OUT_N = 2 * SHARD + 2 * L + 16  # rA | rB | cs11 | csX | s12


def _dma(nc, out, in_):
    nc.sync.dma_start(out=out, in_=in_)


def _proj_pass(nc, P, jobs, after_chunk=None):
    """Projection, chunk-interleaved:
    zt = W2 @ elu(W1 @ X.T + b1) + b2 ([128,2,L] bf16 each)."""
    xp, hs, pph, ppz, w1s, w2s, b1s, b1p, b2s = P
    xts = {}
    for oc in range(L // XCHUNK):
        for j, (xdram, zt) in enumerate(jobs):
            xt = xp.tile([128, 2, XCHUNK], BF16, name="xt", tag=f"xt{j}")
            for dt in range(2):
                _dma(nc, xt[:, dt, :],
                     xdram[dt, :, oc * XCHUNK:(oc + 1) * XCHUNK])
            xts[j] = xt
        for ic in range(XCHUNK // 512):
            c = oc * (XCHUNK // 512) + ic
            cs = slice(c * 512, (c + 1) * 512)
            ics = slice(ic * 512, (ic + 1) * 512)
            for j, (xdram, zt) in enumerate(jobs):
                xt = xts[j]
                hp = pph.tile([128, 2, 512], F32, name="hp", tag="hp")
                for pt in range(2):
                    for dt in range(2):
                        nc.tensor.matmul(
                            hp[:, pt, :],
                            lhsT=w1s[dt][:, pt * 128:(pt + 1) * 128],
                            rhs=xt[:, dt, ics],
                            start=(dt == 0), stop=(dt == 1),
                        )
                # elu(v)+1 = min(exp(v), relu(v)+1), v = hp + b1; the -1 is
                # folded into b2' = b2 - W2.sum(1) on the host.
                e_sb = hs.tile([128, 2, 512], BF16, name="e_sb", tag="e_sb")
                r_sb = hs.tile([128, 2, 512], BF16, name="r_sb", tag="r_sb")
                h_sb = hs.tile([128, 2, 512], BF16, name="h_sb", tag="h_sb")
                for pt in range(2):
                    nc.scalar.activation(e_sb[:, pt, :], hp[:, pt, :], AF.Exp,
                                         bias=b1s[:, pt:pt + 1], scale=1.0)
                    nc.vector.tensor_scalar(out=r_sb[:, pt, :],
                                            in0=hp[:, pt, :],
                                            scalar1=b1p[:, pt:pt + 1],
                                            scalar2=1.0,
                                            op0=ALU.add, op1=ALU.max)
                nc.vector.tensor_tensor(out=h_sb[:, :, :], in0=e_sb[:, :, :],
                                        in1=r_sb[:, :, :], op=ALU.min)
                zp = ppz.tile([128, 2, 512], F32, name="zp", tag="zp")
                for dt in range(2):
                    for k in range(2):
                        nc.tensor.matmul(
                            zp[:, dt, :],
                            lhsT=w2s[k][:, dt * 128:(dt + 1) * 128],
                            rhs=h_sb[:, k, :],
                            start=(k == 0), stop=(k == 1),
                        )
                if True:
                    nc.vector.tensor_scalar(out=zt[:, 0, cs],
                                            in0=zp[:, 0, :],
                                            scalar1=b2s[:, 0:1],
                                            scalar2=None, op0=ALU.add)
                    nc.scalar.activation(zt[:, 1, cs], zp[:, 1, :],
                                         AF.Identity, bias=b2s[:, 1:2],
                                         scale=1.0)
                if after_chunk is not None:
                    after_chunk(c)


def _norm_a(nc, hs, zt, nsf, u, ns_alloc, ones_bf):
    """Norm pass A for one 1024-wide unit (no ScalarE work -- interleaves
    into the projection): squares (Pool, bf16) -> ones-matmul column sums
    -> staged to nsf."""
    cs = slice(u * 1024, (u + 1) * 1024)
    sq = hs.tile([128, 2, 1024], BF16, name="sq", tag="sq")
    nc.gpsimd.tensor_mul(sq[:, :, :], zt[:, :, cs], zt[:, :, cs])
    ns_ps = ns_alloc()
    for half in range(2):
        hs_ = slice(half * 512, (half + 1) * 512)
        for dt in range(2):
            nc.tensor.matmul(ns_ps[0:1, hs_], lhsT=ones_bf[:, :],
                             rhs=sq[:, dt, hs_],
                             start=(dt == 0), stop=(dt == 1))
    nc.vector.tensor_copy(nsf[0:1, cs], ns_ps[:, :])


def _norm_b(nc, P, zt, zb, rlnf, nsf, s12ctx, ones_bf, ones_row):
    """Norm pass B: ONE Ln over [1, L] (batched so the activation-table
    loader doesn't thrash between exp and ln tables), then per 1024-unit
    the rnorm broadcast via K=1 matmuls with exp(-rln/2) fused into the
    PSUM->SBUF copy and zb = zt * rnorm in fp8.  With s12ctx=(zt1, rln1f,
    s12parts): z1.z2 dots for the core's own rows (units 0..1)."""
    hs, nsp, bcp, stp = P
    nc.scalar.activation(rlnf[0:1, :], nsf[0:1, :], AF.Ln)
    for u in range(NMC // 2):
        cs = slice(u * 1024, (u + 1) * 1024)
        bc = bcp.tile([128, 1024], F32, name="bc", tag="bcp")
        for half in range(2):
            hs_ = slice(u * 1024 + half * 512, u * 1024 + (half + 1) * 512)
            nc.tensor.matmul(bc[:, half * 512:(half + 1) * 512],
                             lhsT=ones_row[:, :],
                             rhs=rlnf[0:1, hs_], start=True, stop=True)
        bcs = stp.tile([128, 1024], BF16, name="bcs", tag="bcs")
        nc.scalar.activation(bcs[:, :], bc[:, :], AF.Exp, scale=-0.5)
        nc.vector.tensor_tensor(
            out=zb[:, :, cs], in0=zt[:, :, cs],
            in1=bcs[:, None, :].broadcast_to([128, 2, 1024]),
            op=ALU.mult)
        # z1.z2 -> s12 for the core's own rows (units 0..1)
        if s12ctx is not None and u < NMC // 4:
            zt1, rln1f, s12parts = s12ctx
            prod = hs.tile([128, 2, 1024], BF16, name="prod", tag="sq")
            nc.gpsimd.tensor_mul(prod[:, :, :], zt1[:, :, cs], zt[:, :, cs])
            u_ps = nsp.tile([1, 1024], F32, name="u_ps", tag="nsp")
            for half in range(2):
                hs_ = slice(half * 512, (half + 1) * 512)
                for dt in range(2):
                    nc.tensor.matmul(u_ps[0:1, hs_], lhsT=ones_bf[:, :],
                                     rhs=prod[:, dt, hs_],
                                     start=(dt == 0), stop=(dt == 1))
            t0 = stp.tile([1, 1024], F32, name="t0", tag="t0")
            nc.vector.tensor_tensor(out=t0[:, :], in0=rln1f[0:1, cs],
                                    in1=rlnf[0:1, cs], op=ALU.add)
            nc.scalar.activation(t0[:, :], t0[:, :], AF.Exp, scale=-0.5)
            t1 = stp.tile([1, 1024], F32, name="t1", tag="t1")
            nc.vector.tensor_tensor(out=t1[:, :], in0=u_ps[:, :],
                                    in1=t0[:, :], op=ALU.mult)
            nc.vector.tensor_reduce(out=s12parts[0:1, u:u + 1],
                                    in_=t1[:, :],
                                    axis=mybir.AxisListType.X, op=ALU.add)


def _mm_region(nc, ps, lhs, rhs, t, col0, w):
    """DoubleRow matmuls for one [128, w] region starting at column col0."""
    ts_ = slice(t * 128, (t + 1) * 128)
    for off in range(0, w, 512):
        n = min(512, w - off)
        nc.tensor.matmul(
            ps[:, off:off + n],
            lhsT=lhs[:, :, ts_], rhs=rhs[:, :, col0 + off:col0 + off + n],
            start=True, stop=True, perf_mode=DR,
        )


def _sym_regions(nc, psum_pool, ebpool, rpart, colacc, zb, t,
                 grain=GRAIN, npiece=2):
    """Symmetric gram tile row t: region A = cols [t*128, 2048), region B =
    cols [2048+t*128, 4096), in pieces of `grain`.  exp row-sums via accum
    (piece k of region r -> rpart col npiece*(2*t+r)+k); all but the leading
    diagonal / anti-diagonal 128-col tile also feed colacc (transposed row
    sums)."""
    w = SHARD - t * 128
    for r, col0 in ((0, t * 128), (1, SHARD + t * 128)):
        for k, off in enumerate(range(0, w, grain)):
            pw = min(grain, w - off)
            ps = psum_pool.tile([128, grain], F32, name="ps",
                                tag=f"ps{grain}")
            _mm_region(nc, ps, zb, zb, t, col0 + off, pw)
            eb = ebpool.tile([128, GRAIN], BF16, name="eb", tag="eb")
            idx = npiece * (2 * t + r) + k
            nc.scalar.activation(eb[:, 0:pw], ps[:, 0:pw], AF.Exp, scale=2.0,
                                 accum_out=rpart[:, idx:idx + 1])
            lo = 128 if off == 0 else 0
            if pw > lo:
                ca = slice(col0 + off + lo, col0 + off + pw)
                nc.vector.tensor_tensor(out=colacc[:, ca], in0=colacc[:, ca],
                                        in1=eb[:, lo:pw], op=ALU.add)


def _e12_tile(nc, psum_pool, ebpool, rpart, colacc, zb1, zb2, t, g):
    """One [128, GRAIN] block of E12 (not symmetric): exp row-sums via
    accum; every column feeds colacc."""
    ps = psum_pool.tile([128, GRAIN], F32, name="ps", tag=f"ps{GRAIN}")
    _mm_region(nc, ps, zb1, zb2, t, g * GRAIN, GRAIN)
    eb = ebpool.tile([128, GRAIN], BF16, name="eb", tag="eb")
    nc.scalar.activation(eb[:, :], ps[:, :], AF.Exp, scale=2.0,
                         accum_out=rpart[:, 2 * t + g:2 * t + g + 1])
    gs = slice(g * GRAIN, (g + 1) * GRAIN)
    nc.vector.tensor_tensor(out=colacc[:, gs], in0=colacc[:, gs],
                            in1=eb[:, :], op=ALU.add)


def _build_bass(loop_reps=None, phases=(1, 1), fence=None):
    if fence is None:
        fence = loop_reps is not None
    nc = bacc.Bacc("TRN2", target_bir_lowering=False, debug=False,
                   num_devices=NCORES)
    xin = nc.dram_tensor("xin", [2, 2, 128, L], BF16,
                         kind="ExternalInput").ap()
    wz = nc.dram_tensor("wz", [2, 2, 128, D], BF16, kind="ExternalInput").ap()
    bz = nc.dram_tensor("bz", [2, 2, 128], F32, kind="ExternalInput").ap()
    outp = nc.dram_tensor("outp", [OUT_N], F32, kind="ExternalOutput").ap()
    aps = (xin, wz, bz, outp)

    with tile.TileContext(nc) as tc:
        if phases == "dmaonly":
            def dma_body():
                with tc.tile_pool(name="xp0", bufs=2) as xp0:
                    for inp in range(2):
                        for oc in range(L // XCHUNK):
                            xt = xp0.tile([128, 2, XCHUNK], BF16, name="xt",
                                          tag="xt")
                            for dt in range(2):
                                _dma(nc, xt[:, dt, :],
                                     xin[inp, dt, :,
                                         oc * XCHUNK:(oc + 1) * XCHUNK])
                            nc.vector.tensor_copy(xt[0:1, 0, 0:4],
                                                  xt[0:1, 1, 0:4])
                    st = xp0.tile([1, OUT_N], F32, name="sto")
                    nc.vector.memset(st, 1.0)
                    _dma(nc, outp[:].rearrange("(o n) -> o n", o=1), st[:, :])
            if loop_reps is None:
                dma_body()
            else:
                with tc.For_i(0, loop_reps, 1):
                    dma_body()
        elif loop_reps is None:
            _emit_body(nc, tc, aps, phases, fence=fence)
        else:
            with tc.For_i(0, loop_reps, 1):
                _emit_body(nc, tc, aps, phases, fence=fence)

    nc.compile()
    return nc


def _emit_body(nc, tc, aps, phases=(1, 1), fence=False):
    do_proj, do_sims = phases
    xin, wz, bz, outp = aps
    with (
        tc.tile_pool(name="fencep", bufs=1) as fencep,
        tc.tile_pool(name="consts", bufs=1) as consts,
        tc.tile_pool(name="zbig", bufs=1) as zbig,
        tc.tile_pool(name="accpool", bufs=1) as accpool,
        tc.tile_pool(name="ebpool", bufs=3) as ebpool,
    ):
        # constants
        w1s = [consts.tile([128, D], BF16, name=f"w1_{dt}") for dt in range(2)]
        w2s = [consts.tile([128, D], BF16, name=f"w2_{dt}") for dt in range(2)]
        ftile = None
        if fence:
            ftile = fencep.tile([1, 16], BF16, name="fence")
            nc.vector.memset(ftile, 0.0)
            nc.vector.tensor_copy(w1s[0][0:1, 0:4], ftile[0:1, 0:4])
        for dt in range(2):
            _dma(nc, w1s[dt][:, :], wz[0, dt])
            _dma(nc, w2s[dt][:, :], wz[1, dt])
        b1s = consts.tile([128, 2], F32, name="b1s")
        b2s = consts.tile([128, 2], F32, name="b2s")
        b1p = consts.tile([128, 2], F32, name="b1p")
        for pt in range(2):
            _dma(nc, b1s[:, pt:pt + 1],
                 bz[0, pt].rearrange("(p o) -> p o", o=1))
            _dma(nc, b2s[:, pt:pt + 1],
                 bz[1, pt].rearrange("(p o) -> p o", o=1))
        nc.vector.tensor_scalar(out=b1p[:, :], in0=b1s[:, :], scalar1=1.0,
                                scalar2=None, op0=ALU.add)
        ones_bf = consts.tile([128, 1], BF16, name="ones_bf")
        nc.vector.memset(ones_bf, 1.0)
        ones_row = consts.tile([1, 128], F32, name="ones_row")
        nc.vector.memset(ones_row, 1.0)

        # persistent: normalized fp8 z's + sim accumulators
        zb1 = zbig.tile([128, 2, L], F8, name="zb1")
        zb2 = zbig.tile([128, 2, L], F8, name="zb2")
        rn1f = zbig.tile([1, L], F32, name="rn1f")
        rln2f = zbig.tile([1, L], F32, name="rln2f")
        nsf = zbig.tile([1, L], F32, name="nsf")
        cacc11 = accpool.tile([128, L], BF16, name="cacc11")
        caccX = accpool.tile([128, L], BF16, name="caccX")
        nc.vector.memset(cacc11, 0.0)
        nc.vector.memset(caccX, 0.0)
        rparts = {11: accpool.tile([128, 4 * NT], F32, name="rp11"),
                  12: accpool.tile([128, 2 * NT], F32, name="rp12"),
                  22: accpool.tile([128, 4 * NT], F32, name="rp22")}
        for m in (11, 22):
            nc.vector.memset(rparts[m][:, :], 0.0)
        s12parts = accpool.tile([1, NMC // 2], F32, name="s12parts")
        nc.vector.memset(s12parts, 0.0)
        extra_sinks = []

        with tc.tile_pool(name="zkeep", bufs=1) as zkeep:
            if do_proj:
                zt1 = zkeep.tile([128, 2, L], BF16, name="zt1")
                zt2 = zkeep.tile([128, 2, L], BF16, name="zt2")
                with (
                    tc.tile_pool(name="xp", bufs=2) as xp,
                    tc.tile_pool(name="hs", bufs=2) as hs,
                ):
                    # ---- proj1 with norm1-A interleaved ----
                    with (
                        tc.tile_pool(name="pph", bufs=2, space="PSUM") as pph,
                        tc.tile_pool(name="ppz", bufs=1, space="PSUM") as ppz,
                        tc.tile_pool(name="nsp", bufs=1, space="PSUM") as nsp,
                    ):
                        P = (xp, hs, pph, ppz, w1s, w2s, b1s, b1p, b2s)

                        def na1(c):
                            if c % 2 == 1:
                                _norm_a(nc, hs, zt1, nsf, c // 2,
                                        lambda: nsp.tile([1, 1024], F32,
                                                         name="ns",
                                                         tag="nsp"),
                                        ones_bf)
                        _proj_pass(nc, P, [(xin[0], zt1)], after_chunk=na1)
                    # ---- norm1-B ----
                    with (
                        tc.tile_pool(name="nspb", bufs=2,
                                     space="PSUM") as nspb,
                        tc.tile_pool(name="bcp", bufs=2, space="PSUM") as bcp,
                        tc.tile_pool(name="stp", bufs=2) as stp,
                    ):
                        PN = (hs, nspb, bcp, stp)
                        _norm_b(nc, PN, zt1, zb1, rn1f, nsf,
                                None, ones_bf, ones_row)
                    # ---- proj2 with E11 + norm2-A interleaved ----
                    with tc.tile_pool(name="psA", bufs=2,
                                      space="PSUM") as psA:
                        with (
                            tc.tile_pool(name="pph2", bufs=1,
                                         space="PSUM") as pph2,
                            tc.tile_pool(name="ppz2", bufs=1,
                                         space="PSUM") as ppz2,
                        ):
                            P2 = (xp, hs, pph2, ppz2, w1s, w2s, b1s, b1p,
                                  b2s)

                            def after2(c):
                                if do_sims:
                                    for t in (2 * c, 2 * c + 1):
                                        _sym_regions(nc, psA, ebpool,
                                                     rparts[11], cacc11,
                                                     zb1, t, grain=1024)
                                if c % 2 == 1:
                                    _norm_a(nc, hs, zt2, nsf, c // 2,
                                            lambda: psA.tile(
                                                [128, 1024], F32,
                                                name="psa",
                                                tag="ps1024")[0:1, 0:1024],
                                            ones_bf)
                            _proj_pass(nc, P2, [(xin[1], zt2)],
                                       after_chunk=after2)
                        # ---- norm2-B (+ s12) while E11 drains ----
                        with (
                            tc.tile_pool(name="nsp2", bufs=1,
                                         space="PSUM") as nsp2,
                            tc.tile_pool(name="bcp2", bufs=1,
                                         space="PSUM") as bcp2,
                            tc.tile_pool(name="stp2", bufs=2) as stp2,
                        ):
                            PN2 = (hs, nsp2, bcp2, stp2)
                            _norm_b(nc, PN2, zt2, zb2, rln2f, nsf,
                                    (zt1, rn1f, s12parts), ones_bf,
                                    ones_row)
                    # cacc11 is complete: stream its column sums out now
                    with tc.tile_pool(name="csp1", bufs=2,
                                      space="PSUM") as csp1:
                        cst11 = zbig.tile([1, L], F32, name="cst11")
                        for c in range(NMC):
                            cs = slice(c * 512, (c + 1) * 512)
                            ps = csp1.tile([1, 512], F32, name="c1ps",
                                           tag="c1ps")
                            nc.tensor.matmul(ps[:, :], lhsT=ones_bf[:, :],
                                             rhs=cacc11[:, cs], start=True,
                                             stop=True)
                            nc.vector.tensor_copy(cst11[0:1, cs], ps[:, :])
                        _dma(nc, outp[4096:4096 + L]
                             .rearrange("(o l) -> o l", o=1), cst11[:, :])
                        extra_sinks.append(cst11)
            else:
                nc.vector.memset(zb1[:, :, :], 0.06)
                nc.vector.memset(zb2[:, :, :], 0.06)
                nc.vector.memset(s12parts[:, :], 1.0)
                nc.vector.memset(rn1f[:, :], 1.0)
                if do_sims:
                    with tc.tile_pool(name="psA0", bufs=2,
                                      space="PSUM") as psA0:
                        for t in range(NT):
                            _sym_regions(nc, psA0, ebpool, rparts[11],
                                         cacc11, zb1, t, grain=1024)

        # ---------------- E12 + E22 sims ----------------
        with tc.tile_pool(name="simpsum", bufs=2, space="PSUM") as simpsum:
            if do_sims:
                for t in range(NT):
                    for g in range(L // GRAIN):
                        _e12_tile(nc, simpsum, ebpool, rparts[12], caccX,
                                  zb1, zb2, t, g)
                    _sym_regions(nc, simpsum, ebpool, rparts[22], caccX,
                                 zb2, t)
            else:
                for m in (11, 12, 22):
                    nc.vector.memset(rparts[m][:, :], 1.0)

        # final reductions + packed store
        with tc.tile_pool(name="outpool", bufs=1) as outpool:
            rfin = {}
            for mat in (11, 12, 22):
                nsub = 2 if mat == 12 else 4
                rfin[mat] = outpool.tile([128, NT], F32, name=f"rf{mat}")
                nc.vector.tensor_reduce(
                    out=rfin[mat][:, :],
                    in_=rparts[mat][:, :].rearrange("p (t h) -> p t h",
                                                    h=nsub),
                    axis=mybir.AxisListType.X, op=ALU.add)
            rA = outpool.tile([128, NT], F32, name="rA")
            nc.vector.tensor_tensor(out=rA[:, :], in0=rfin[11][:, :],
                                    in1=rfin[12][:, :], op=ALU.add)
            _dma(nc, outp[0:2048].rearrange("(p t) -> p t", t=NT), rA[:, :])
            _dma(nc, outp[2048:4096].rearrange("(p t) -> p t", t=NT),
                 rfin[22][:, :])
            # column sums of caccX (over partitions); cacc11's went out early
            with tc.tile_pool(name="cspsum", bufs=4, space="PSUM") as cspsum:
                stX = outpool.tile([1, L], F32, name="cstX")
                for c in range(NMC):
                    cs = slice(c * 512, (c + 1) * 512)
                    ps = cspsum.tile([1, 512], F32, name="csps", tag="csps")
                    nc.tensor.matmul(ps[:, :], lhsT=ones_bf[:, :],
                                     rhs=caccX[:, cs], start=True, stop=True)
                    if c % 2 == 0:
                        nc.vector.tensor_copy(stX[0:1, cs], ps[:, :])
                    else:
                        nc.scalar.copy(stX[0:1, cs], ps[:, :])
                _dma(nc, outp[4096 + L:4096 + 2 * L]
                     .rearrange("(o l) -> o l", o=1), stX[:, :])
            # s12 scalar
            st16 = outpool.tile([1, 16], F32, name="st16")
            nc.vector.memset(st16, 0.0)
            nc.vector.tensor_reduce(out=st16[0:1, 0:1], in_=s12parts[:, :],
                                    axis=mybir.AxisListType.X, op=ALU.add)
            _dma(nc, outp[4096 + 2 * L:OUT_N].rearrange("(o l) -> o l", o=1),
                 st16[:, :])
            if fence:
                sinks = [st16, rA, rfin[22], stX] + extra_sinks
                for i, s in enumerate(sinks):
                    nc.vector.tensor_copy(ftile[0:1, i:i + 1], s[0:1, 0:1])


_NC_CACHE = None


def _get_nc():
    global _NC_CACHE
    if _NC_CACHE is None:
        _NC_CACHE = _build_bass()
    return _NC_CACHE


class _Runner:
    """jit-once SPMD runner (mirrors bass2jax.run_bass_via_pjrt multi-core)."""

    def __init__(self, nc):
        import jax
        import jax.numpy as jnp
        from jax.sharding import Mesh, PartitionSpec, NamedSharding
        from jax.experimental.shard_map import shard_map
        from concourse import bass2jax
        import concourse.mybir as _mybir

        bass2jax.install_neuronx_cc_hook()
        self.jax = jax
        in_names, out_names, out_avals = [], [], []
        partition_name = (nc.partition_id_tensor.name
                          if nc.partition_id_tensor else None)
        for alloc in nc.m.functions[0].allocations:
            if not isinstance(alloc, _mybir.MemoryLocationSet):
                continue
            name = alloc.memorylocations[0].name
            if alloc.kind == "ExternalInput":
                if name != partition_name:
                    in_names.append(name)
            elif alloc.kind == "ExternalOutput":
                out_names.append(name)
                out_avals.append(jax.core.ShapedArray(
                    tuple(alloc.tensor_shape), _mybir.dt.np(alloc.dtype)))
        self.in_names, self.out_names, self.out_avals = (
            in_names, out_names, out_avals)
        n_params, n_outs = len(in_names), len(out_names)
        all_names = in_names + out_names
        if partition_name is not None:
            all_names.append(partition_name)

        def _body(*args):
            operands = list(args)
            if partition_name is not None:
                operands.append(bass2jax.partition_id_tensor())
            return tuple(bass2jax._bass_exec_p.bind(
                *operands, out_avals=tuple(out_avals),
                in_names=tuple(all_names), out_names=tuple(out_names),
                lowering_input_output_aliases=(),
                sim_require_finite=True, sim_require_nnan=True, nc=nc))

        devices = jax.devices()[:NCORES]
        self.mesh = Mesh(np.asarray(devices), ("core",))
        self.spec = PartitionSpec("core")
        in_specs = (self.spec,) * (n_params + n_outs)
        out_specs = (self.spec,) * n_outs
        self.fn = jax.jit(shard_map(_body, mesh=self.mesh, in_specs=in_specs,
                                    out_specs=out_specs, check_rep=False),
                          keep_unused=True)
        self.n_params, self.n_outs = n_params, n_outs
        sh = NamedSharding(self.mesh, self.spec)
        shapes = [(NCORES * a.shape[0], *a.shape[1:]) for a in out_avals]
        self._zeros_fn = jax.jit(
            lambda: tuple(jnp.zeros(s, a.dtype)
                          for s, a in zip(shapes, out_avals)),
            out_shardings=tuple(sh for _ in out_avals))

    def put_inputs(self, in_maps):
        import jax
        from jax.sharding import NamedSharding
        sh = NamedSharding(self.mesh, self.spec)
        concat = [np.concatenate([np.asarray(m[n]) for m in in_maps], axis=0)
                  for n in self.in_names]
        return [jax.device_put(a, sh) for a in concat]

    def make_zeros(self):
        return list(self._zeros_fn())

    def run(self, dev_inputs, dev_zeros):
        outs = self.fn(*dev_inputs, *dev_zeros)
        self.jax.block_until_ready(outs)
        return outs

    def results(self, outs):
        res = []
        for c in range(NCORES):
            res.append({
                n: np.asarray(outs[i]).reshape(
                    NCORES, *self.out_avals[i].shape)[c]
                for i, n in enumerate(self.out_names)})
        return res


_RUNNER = None


def _get_runner():
    global _RUNNER
    if _RUNNER is None:
        _RUNNER = _Runner(_get_nc())
    return _RUNNER


def _make_in_maps(X1, X2, W1, b1, W2, b2):
    import ml_dtypes
    bf = ml_dtypes.bfloat16
    wzv = np.stack([
        np.ascontiguousarray(W1.T).reshape(2, 128, D),
        np.ascontiguousarray(W2.T).reshape(2, 128, D),
    ]).astype(bf)
    # the +1 folded into h flows through the bf16 matmul, so subtract the
    # bf16-rounded row sums to match it exactly
    b2p = b2 - W2.astype(bf).astype(np.float64).sum(axis=1)
    bzv = np.stack([b1.reshape(2, 128), b2p.reshape(2, 128)]
                   ).astype(np.float32)
    in_maps = []
    for c in range(NCORES):
        b, s = divmod(c, 2)
        x1 = np.roll(np.ascontiguousarray(X1[b].T), -s * SHARD, axis=1)
        x2 = np.roll(np.ascontiguousarray(X2[b].T), -s * SHARD, axis=1)
        xinv = np.stack([x1.reshape(2, 128, L), x2.reshape(2, 128, L)]
                        ).astype(bf)
        in_maps.append({"xin": xinv, "wz": wzv, "bz": bzv})
    return in_maps


def _finish_host(results):
    """Combine per-core partials into the final scalar loss (float64)."""
    e2 = np.exp(2.0)
    total = 0.0
    for b in range(B):
        o0 = results[2 * b]["outp"].astype(np.float64)
        o1 = results[2 * b + 1]["outp"].astype(np.float64)

        def rpart(o, lo):
            return o[lo:lo + 2048].reshape(128, NT).T.reshape(-1)

        den1 = np.concatenate([
            rpart(o0, 0) + o0[4096:4096 + 2048] + o1[4096 + 2048:4096 + L],
            rpart(o1, 0) + o1[4096:4096 + 2048] + o0[4096 + 2048:4096 + L],
        ]) - e2
        cX0, cX1 = o0[4096 + L:4096 + 2 * L], o1[4096 + L:4096 + 2 * L]
        den2 = np.concatenate([
            rpart(o0, 2048) + cX0[0:2048] + cX1[2048:L],
            rpart(o1, 2048) + cX1[0:2048] + cX0[2048:L],
        ]) - e2
        s12t = o0[4096 + 2 * L] + o1[4096 + 2 * L]
        l1 = np.sum(np.log(den1)) - 2.0 * s12t
        l2 = np.sum(np.log(den2)) - 2.0 * s12t
        total += 0.5 * (l1 + l2)
    return np.float32(total / B)


def kernel(X1, X2, W1, b1, W2, b2):
    X1 = np.asarray(X1, dtype=np.float32)
    X2 = np.asarray(X2, dtype=np.float32)
    W1 = np.asarray(W1, dtype=np.float32)
    b1 = np.asarray(b1, dtype=np.float32)
    W2 = np.asarray(W2, dtype=np.float32)
    b2 = np.asarray(b2, dtype=np.float32)
    r = _get_runner()
    in_maps = _make_in_maps(X1, X2, W1, b1, W2, b2)
    outs = r.run(r.put_inputs(in_maps), r.make_zeros())
    return _finish_host(r.results(outs))


# revision 36
# speedup vs baseline: 1.0103x; 1.0103x over previous
"""Trainium2 Bass kernel for the ContrastiveModel loss.

Math (per batch b):
    z1 = proj(X1[b]), z2 = proj(X2[b]);  proj(x) = elu(x@W1.T+b1)@W2.T+b2
    z1n, z2n = L2-normalized rows
    E11 = exp(z1n z1n^T / tau), E12 = exp(z1n z2n^T / tau), E22 likewise
    l1 = sum_l [log(rowsum(E11)+rowsum(E12)-e^2) - 2*s12[l]]
    l2 = sum_l [log(rowsum(E22)+colsum(E12)-e^2) - 2*s12[l]]
    loss = mean_b 0.5*(l1+l2),  s12[l] = z1n[l].z2n[l]

Sharding: 8 cores, 2 per batch; each core owns a 2048-row block of the sim
matrices in a rolled coordinate frame (host rolls the L axis by the shard
offset so one SPMD program serves all cores; rolled rows 0..2048 are the
core's own rows, rolled cols 2048..4096 the partner's).

E11/E22 are symmetric: each unordered tile pair is computed once.  Per
128-row tile t the core computes region A = cols [t*128, 2048) and region
B = cols [2048+t*128, 4096).  The leading 128-col tile of each region (the
diagonal / anti-diagonal tile) contributes row sums only; every later tile
contributes its row sums via the activation accum and its transposed row
sums via a column accumulator.  Host combine: den rows of core s =
accum_s + colacc_s[0:2048] + colacc_partner[2048:4096].  The E22 and E12
column accumulators share one buffer (both feed den2); E11 has its own
(den1).  E12 is not symmetric and is computed in full.

Pipeline per core (software-pipelined for ScalarE occupancy -- on HW the
activation engine runs faster than the cost model, so hiding E11 under the
second projection measures faster than the phase-sequential layout):
  1. proj1 with norm1-A interleaved per chunk (squares on Pool, ones-matmul
     column sums staged to SBUF -- no ScalarE work), then norm1-B: one
     batched Ln over [1,L] (avoids activation-table thrash) and
     rnorm = exp(-ln(|z|^2)/2) fused into the broadcast PSUM->SBUF copy;
     zb1 = zt1 * rnorm in fp8e4m3.  elu(v)+1 = min(exp(v), relu(v)+1) with
     the -1 folded into b2' = b2 - W2.sum(1) on the host (bf16-rounded W2
     to match the device matmul).
  2. proj2 with E11 sim tiles (grain 1024) AND norm2-A interleaved per
     chunk; then norm2-B (+ on-chip s12 scalar) while E11 drains; cacc11's
     column sums stream out right after.
  3. E12 + E22 sims: fp8 DoubleRow matmuls (K=256 per instruction, 0.5
     cyc/row) fill [128,2048] PSUM blocks; ScalarE exp(2x) with fused
     row-sum (accum_out); column sums accumulate on DVE (bf16 2x).
  4. one packed output: [rA | rB | cs11 | csX | s12].
"""

import numpy as np

import concourse.bass as bass
import concourse.mybir as mybir
import concourse.tile as tile
from concourse import bacc

F32 = mybir.dt.float32
BF16 = mybir.dt.bfloat16
F8 = mybir.dt.float8e4
AF = mybir.ActivationFunctionType
ALU = mybir.AluOpType
DR = mybir.MatmulPerfMode.DoubleRow

B, L, D = 4, 4096, 256
NCORES = 8
SHARD = L // 2            # rows of the sim matrices per core
NT = SHARD // 128         # 16 l-tiles per core
NMC = L // 512            # 8 chunks of 512
XCHUNK = 1024             # x DMA burst width
GRAIN = 2048              # E12 column-group width (4 PSUM banks)
OUT_N = 2 * SHARD + 2 * L + 16  # rA | rB | cs11 | csX | s12


def _dma(nc, out, in_):
    nc.sync.dma_start(out=out, in_=in_)


def _proj_pass(nc, P, jobs, after_chunk=None):
    """Projection, chunk-interleaved:
    zt = W2 @ elu(W1 @ X.T + b1) + b2 ([128,2,L] bf16 each)."""
    xp, hs, pph, ppz, w1s, w2s, b1s, b1p, b2s = P
    xts = {}
    for oc in range(L // XCHUNK):
        for j, (xdram, zt) in enumerate(jobs):
            xt = xp.tile([128, 2, XCHUNK], BF16, name="xt", tag=f"xt{j}")
            for dt in range(2):
                _dma(nc, xt[:, dt, :],
                     xdram[dt, :, oc * XCHUNK:(oc + 1) * XCHUNK])
            xts[j] = xt
        for ic in range(XCHUNK // 512):
            c = oc * (XCHUNK // 512) + ic
            cs = slice(c * 512, (c + 1) * 512)
            ics = slice(ic * 512, (ic + 1) * 512)
            for j, (xdram, zt) in enumerate(jobs):
                xt = xts[j]
                hp = pph.tile([128, 2, 512], F32, name="hp", tag="hp")
                for pt in range(2):
                    for dt in range(2):
                        nc.tensor.matmul(
                            hp[:, pt, :],
                            lhsT=w1s[dt][:, pt * 128:(pt + 1) * 128],
                            rhs=xt[:, dt, ics],
                            start=(dt == 0), stop=(dt == 1),
                        )
                # elu(v)+1 = min(exp(v), relu(v)+1), v = hp + b1; the -1 is
                # folded into b2' = b2 - W2.sum(1) on the host.
                e_sb = hs.tile([128, 2, 512], BF16, name="e_sb", tag="e_sb")
                r_sb = hs.tile([128, 2, 512], BF16, name="r_sb", tag="r_sb")
                h_sb = hs.tile([128, 2, 512], BF16, name="h_sb", tag="h_sb")
                for pt in range(2):
                    nc.scalar.activation(e_sb[:, pt, :], hp[:, pt, :], AF.Exp,
                                         bias=b1s[:, pt:pt + 1], scale=1.0)
                    nc.vector.tensor_scalar(out=r_sb[:, pt, :],
                                            in0=hp[:, pt, :],
                                            scalar1=b1p[:, pt:pt + 1],
                                            scalar2=1.0,
                                            op0=ALU.add, op1=ALU.max)
                nc.vector.tensor_tensor(out=h_sb[:, :, :], in0=e_sb[:, :, :],
                                        in1=r_sb[:, :, :], op=ALU.min)
                zp = ppz.tile([128, 2, 512], F32, name="zp", tag="zp")
                for dt in range(2):
                    for k in range(2):
                        nc.tensor.matmul(
                            zp[:, dt, :],
                            lhsT=w2s[k][:, dt * 128:(dt + 1) * 128],
                            rhs=h_sb[:, k, :],
                            start=(k == 0), stop=(k == 1),
                        )
                if True:
                    nc.vector.tensor_scalar(out=zt[:, 0, cs],
                                            in0=zp[:, 0, :],
                                            scalar1=b2s[:, 0:1],
                                            scalar2=None, op0=ALU.add)
                    nc.scalar.activation(zt[:, 1, cs], zp[:, 1, :],
                                         AF.Identity, bias=b2s[:, 1:2],
                                         scale=1.0)
                if after_chunk is not None:
                    after_chunk(c)


def _norm_a(nc, hs, zt, nsf, c, ns_alloc, ones_bf):
    """Norm pass A for one 512-chunk (no ScalarE work -- interleaves into
    the projection): squares (Pool, bf16) -> ones-matmul column sums ->
    staged to nsf."""
    cs = slice(c * 512, (c + 1) * 512)
    sq = hs.tile([128, 2, 512], BF16, name="sq", tag="sq")
    nc.gpsimd.tensor_mul(sq[:, :, :], zt[:, :, cs], zt[:, :, cs])
    ns_ps = ns_alloc()
    for dt in range(2):
        nc.tensor.matmul(ns_ps[:, :], lhsT=ones_bf[:, :],
                         rhs=sq[:, dt, :],
                         start=(dt == 0), stop=(dt == 1))
    nc.vector.tensor_copy(nsf[0:1, cs], ns_ps[:, :])


def _norm_b(nc, P, zt, zb, rlnf, nsf, s12ctx, ones_bf, ones_row):
    """Norm pass B: ONE Ln over [1, L] (batched so the activation-table
    loader doesn't thrash between exp and ln tables), then per chunk the
    rnorm broadcast via K=1 matmul with exp(-rln/2) fused into the
    PSUM->SBUF copy and zb = zt * rnorm in fp8.  With s12ctx=(zt1, rln1f,
    s12parts): z1.z2 dots for the core's own rows (chunks 0..3)."""
    hs, nsp, bcp, stp = P
    nc.scalar.activation(rlnf[0:1, :], nsf[0:1, :], AF.Ln)
    for c in range(NMC):
        cs = slice(c * 512, (c + 1) * 512)
        bc = bcp.tile([128, 512], F32, name="bc", tag="bcp")
        nc.tensor.matmul(bc[:, :], lhsT=ones_row[:, :],
                         rhs=rlnf[0:1, cs], start=True, stop=True)
        bcs = stp.tile([128, 512], BF16, name="bcs", tag="bcs")
        nc.scalar.activation(bcs[:, :], bc[:, :], AF.Exp, scale=-0.5)
        nc.vector.tensor_tensor(
            out=zb[:, :, cs], in0=zt[:, :, cs],
            in1=bcs[:, None, :].broadcast_to([128, 2, 512]),
            op=ALU.mult)
        # z1.z2 -> s12 for the core's own rows (chunks 0..3)
        if s12ctx is not None and c < NMC // 2:
            zt1, rln1f, s12parts = s12ctx
            prod = hs.tile([128, 2, 512], BF16, name="prod", tag="sq")
            nc.gpsimd.tensor_mul(prod[:, :, :], zt1[:, :, cs], zt[:, :, cs])
            u_ps = nsp.tile([1, 512], F32, name="u_ps", tag="nsp")
            for dt in range(2):
                nc.tensor.matmul(u_ps[:, :], lhsT=ones_bf[:, :],
                                 rhs=prod[:, dt, :],
                                 start=(dt == 0), stop=(dt == 1))
            t0 = stp.tile([1, 512], F32, name="t0", tag="t0")
            nc.vector.tensor_tensor(out=t0[:, :], in0=rln1f[0:1, cs],
                                    in1=rlnf[0:1, cs], op=ALU.add)
            nc.scalar.activation(t0[:, :], t0[:, :], AF.Exp, scale=-0.5)
            t1 = stp.tile([1, 512], F32, name="t1", tag="t1")
            nc.vector.tensor_tensor(out=t1[:, :], in0=u_ps[:, :],
                                    in1=t0[:, :], op=ALU.mult)
            nc.vector.tensor_reduce(out=s12parts[0:1, c:c + 1],
                                    in_=t1[:, :],
                                    axis=mybir.AxisListType.X, op=ALU.add)


def _mm_region(nc, ps, lhs, rhs, t, col0, w):
    """DoubleRow matmuls for one [128, w] region starting at column col0."""
    ts_ = slice(t * 128, (t + 1) * 128)
    for off in range(0, w, 512):
        n = min(512, w - off)
        nc.tensor.matmul(
            ps[:, off:off + n],
            lhsT=lhs[:, :, ts_], rhs=rhs[:, :, col0 + off:col0 + off + n],
            start=True, stop=True, perf_mode=DR,
        )


def _sym_regions(nc, psum_pool, ebpool, rpart, colacc, zb, t,
                 grain=GRAIN, npiece=2):
    """Symmetric gram tile row t: region A = cols [t*128, 2048), region B =
    cols [2048+t*128, 4096), in pieces of `grain`.  exp row-sums via accum
    (piece k of region r -> rpart col npiece*(2*t+r)+k); all but the leading
    diagonal / anti-diagonal 128-col tile also feed colacc (transposed row
    sums)."""
    w = SHARD - t * 128
    for r, col0 in ((0, t * 128), (1, SHARD + t * 128)):
        for k, off in enumerate(range(0, w, grain)):
            pw = min(grain, w - off)
            ps = psum_pool.tile([128, grain], F32, name="ps",
                                tag=f"ps{grain}")
            _mm_region(nc, ps, zb, zb, t, col0 + off, pw)
            eb = ebpool.tile([128, GRAIN], BF16, name="eb", tag="eb")
            idx = npiece * (2 * t + r) + k
            nc.scalar.activation(eb[:, 0:pw], ps[:, 0:pw], AF.Exp, scale=2.0,
                                 accum_out=rpart[:, idx:idx + 1])
            lo = 128 if off == 0 else 0
            if pw > lo:
                ca = slice(col0 + off + lo, col0 + off + pw)
                nc.vector.tensor_tensor(out=colacc[:, ca], in0=colacc[:, ca],
                                        in1=eb[:, lo:pw], op=ALU.add)


def _e12_tile(nc, psum_pool, ebpool, rpart, colacc, zb1, zb2, t, g):
    """One [128, GRAIN] block of E12 (not symmetric): exp row-sums via
    accum; every column feeds colacc."""
    ps = psum_pool.tile([128, GRAIN], F32, name="ps", tag=f"ps{GRAIN}")
    _mm_region(nc, ps, zb1, zb2, t, g * GRAIN, GRAIN)
    eb = ebpool.tile([128, GRAIN], BF16, name="eb", tag="eb")
    nc.scalar.activation(eb[:, :], ps[:, :], AF.Exp, scale=2.0,
                         accum_out=rpart[:, 2 * t + g:2 * t + g + 1])
    gs = slice(g * GRAIN, (g + 1) * GRAIN)
    nc.vector.tensor_tensor(out=colacc[:, gs], in0=colacc[:, gs],
                            in1=eb[:, :], op=ALU.add)


def _build_bass(loop_reps=None, phases=(1, 1), fence=None):
    if fence is None:
        fence = loop_reps is not None
    nc = bacc.Bacc("TRN2", target_bir_lowering=False, debug=False,
                   num_devices=NCORES)
    xin = nc.dram_tensor("xin", [2, 2, 128, L], BF16,
                         kind="ExternalInput").ap()
    wz = nc.dram_tensor("wz", [2, 2, 128, D], BF16, kind="ExternalInput").ap()
    bz = nc.dram_tensor("bz", [2, 2, 128], F32, kind="ExternalInput").ap()
    outp = nc.dram_tensor("outp", [OUT_N], F32, kind="ExternalOutput").ap()
    aps = (xin, wz, bz, outp)

    with tile.TileContext(nc) as tc:
        if phases == "dmaonly":
            def dma_body():
                with tc.tile_pool(name="xp0", bufs=2) as xp0:
                    for inp in range(2):
                        for oc in range(L // XCHUNK):
                            xt = xp0.tile([128, 2, XCHUNK], BF16, name="xt",
                                          tag="xt")
                            for dt in range(2):
                                _dma(nc, xt[:, dt, :],
                                     xin[inp, dt, :,
                                         oc * XCHUNK:(oc + 1) * XCHUNK])
                            nc.vector.tensor_copy(xt[0:1, 0, 0:4],
                                                  xt[0:1, 1, 0:4])
                    st = xp0.tile([1, OUT_N], F32, name="sto")
                    nc.vector.memset(st, 1.0)
                    _dma(nc, outp[:].rearrange("(o n) -> o n", o=1), st[:, :])
            if loop_reps is None:
                dma_body()
            else:
                with tc.For_i(0, loop_reps, 1):
                    dma_body()
        elif loop_reps is None:
            _emit_body(nc, tc, aps, phases, fence=fence)
        else:
            with tc.For_i(0, loop_reps, 1):
                _emit_body(nc, tc, aps, phases, fence=fence)

    nc.compile()
    return nc


def _emit_body(nc, tc, aps, phases=(1, 1), fence=False):
    do_proj, do_sims = phases
    xin, wz, bz, outp = aps
    with (
        tc.tile_pool(name="fencep", bufs=1) as fencep,
        tc.tile_pool(name="consts", bufs=1) as consts,
        tc.tile_pool(name="zbig", bufs=1) as zbig,
        tc.tile_pool(name="accpool", bufs=1) as accpool,
        tc.tile_pool(name="ebpool", bufs=4) as ebpool,
    ):
        # constants
        w1s = [consts.tile([128, D], BF16, name=f"w1_{dt}") for dt in range(2)]
        w2s = [consts.tile([128, D], BF16, name=f"w2_{dt}") for dt in range(2)]
        ftile = None
        if fence:
            ftile = fencep.tile([1, 16], BF16, name="fence")
            nc.vector.memset(ftile, 0.0)
            nc.vector.tensor_copy(w1s[0][0:1, 0:4], ftile[0:1, 0:4])
        for dt in range(2):
            _dma(nc, w1s[dt][:, :], wz[0, dt])
            _dma(nc, w2s[dt][:, :], wz[1, dt])
        b1s = consts.tile([128, 2], F32, name="b1s")
        b2s = consts.tile([128, 2], F32, name="b2s")
        b1p = consts.tile([128, 2], F32, name="b1p")
        for pt in range(2):
            _dma(nc, b1s[:, pt:pt + 1],
                 bz[0, pt].rearrange("(p o) -> p o", o=1))
            _dma(nc, b2s[:, pt:pt + 1],
                 bz[1, pt].rearrange("(p o) -> p o", o=1))
        nc.vector.tensor_scalar(out=b1p[:, :], in0=b1s[:, :], scalar1=1.0,
                                scalar2=None, op0=ALU.add)
        ones_bf = consts.tile([128, 1], BF16, name="ones_bf")
        nc.vector.memset(ones_bf, 1.0)
        ones_row = consts.tile([1, 128], F32, name="ones_row")
        nc.vector.memset(ones_row, 1.0)

        # persistent: normalized fp8 z's + sim accumulators
        zb1 = zbig.tile([128, 2, L], F8, name="zb1")
        zb2 = zbig.tile([128, 2, L], F8, name="zb2")
        rn1f = zbig.tile([1, L], F32, name="rn1f")
        rln2f = zbig.tile([1, L], F32, name="rln2f")
        nsf = zbig.tile([1, L], F32, name="nsf")
        cacc11 = accpool.tile([128, L], BF16, name="cacc11")
        caccX = accpool.tile([128, L], BF16, name="caccX")
        nc.vector.memset(cacc11, 0.0)
        nc.vector.memset(caccX, 0.0)
        rparts = {11: accpool.tile([128, 4 * NT], F32, name="rp11"),
                  12: accpool.tile([128, 2 * NT], F32, name="rp12"),
                  22: accpool.tile([128, 4 * NT], F32, name="rp22")}
        for m in (11, 22):
            nc.vector.memset(rparts[m][:, :], 0.0)
        s12parts = accpool.tile([1, NMC // 2], F32, name="s12parts")
        extra_sinks = []

        with tc.tile_pool(name="zkeep", bufs=1) as zkeep:
            if do_proj:
                zt1 = zkeep.tile([128, 2, L], BF16, name="zt1")
                zt2 = zkeep.tile([128, 2, L], BF16, name="zt2")
                with (
                    tc.tile_pool(name="xp", bufs=3) as xp,
                    tc.tile_pool(name="hs", bufs=3) as hs,
                ):
                    # ---- proj1 with norm1-A interleaved ----
                    with (
                        tc.tile_pool(name="pph", bufs=2, space="PSUM") as pph,
                        tc.tile_pool(name="ppz", bufs=1, space="PSUM") as ppz,
                        tc.tile_pool(name="nsp", bufs=2, space="PSUM") as nsp,
                    ):
                        P = (xp, hs, pph, ppz, w1s, w2s, b1s, b1p, b2s)

                        def na1(c):
                            _norm_a(nc, hs, zt1, nsf, c,
                                    lambda: nsp.tile([1, 512], F32,
                                                     name="ns", tag="nsp"),
                                    ones_bf)
                        _proj_pass(nc, P, [(xin[0], zt1)], after_chunk=na1)
                    # ---- norm1-B ----
                    with (
                        tc.tile_pool(name="nspb", bufs=2,
                                     space="PSUM") as nspb,
                        tc.tile_pool(name="bcp", bufs=2, space="PSUM") as bcp,
                        tc.tile_pool(name="stp", bufs=3) as stp,
                    ):
                        PN = (hs, nspb, bcp, stp)
                        _norm_b(nc, PN, zt1, zb1, rn1f, nsf,
                                None, ones_bf, ones_row)
                    # ---- proj2 with E11 + norm2-A interleaved ----
                    with tc.tile_pool(name="psA", bufs=2,
                                      space="PSUM") as psA:
                        with (
                            tc.tile_pool(name="pph2", bufs=1,
                                         space="PSUM") as pph2,
                            tc.tile_pool(name="ppz2", bufs=1,
                                         space="PSUM") as ppz2,
                        ):
                            P2 = (xp, hs, pph2, ppz2, w1s, w2s, b1s, b1p,
                                  b2s)

                            def after2(c):
                                if do_sims:
                                    for t in (2 * c, 2 * c + 1):
                                        _sym_regions(nc, psA, ebpool,
                                                     rparts[11], cacc11,
                                                     zb1, t, grain=1024)
                                _norm_a(nc, hs, zt2, nsf, c,
                                        lambda: psA.tile(
                                            [128, 1024], F32, name="psa",
                                            tag="ps1024")[0:1, 0:512],
                                        ones_bf)
                            _proj_pass(nc, P2, [(xin[1], zt2)],
                                       after_chunk=after2)
                        # ---- norm2-B (+ s12) while E11 drains ----
                        with (
                            tc.tile_pool(name="nsp2", bufs=2,
                                         space="PSUM") as nsp2,
                            tc.tile_pool(name="bcp2", bufs=2,
                                         space="PSUM") as bcp2,
                            tc.tile_pool(name="stp2", bufs=3) as stp2,
                        ):
                            PN2 = (hs, nsp2, bcp2, stp2)
                            _norm_b(nc, PN2, zt2, zb2, rln2f, nsf,
                                    (zt1, rn1f, s12parts), ones_bf,
                                    ones_row)
                    # cacc11 is complete: stream its column sums out now
                    with tc.tile_pool(name="csp1", bufs=2,
                                      space="PSUM") as csp1:
                        cst11 = zbig.tile([1, L], F32, name="cst11")
                        for c in range(NMC):
                            cs = slice(c * 512, (c + 1) * 512)
                            ps = csp1.tile([1, 512], F32, name="c1ps",
                                           tag="c1ps")
                            nc.tensor.matmul(ps[:, :], lhsT=ones_bf[:, :],
                                             rhs=cacc11[:, cs], start=True,
                                             stop=True)
                            nc.vector.tensor_copy(cst11[0:1, cs], ps[:, :])
                        _dma(nc, outp[4096:4096 + L]
                             .rearrange("(o l) -> o l", o=1), cst11[:, :])
                        extra_sinks.append(cst11)
            else:
                nc.vector.memset(zb1[:, :, :], 0.06)
                nc.vector.memset(zb2[:, :, :], 0.06)
                nc.vector.memset(s12parts[:, :], 1.0)
                nc.vector.memset(rn1f[:, :], 1.0)
                if do_sims:
                    with tc.tile_pool(name="psA0", bufs=2,
                                      space="PSUM") as psA0:
                        for t in range(NT):
                            _sym_regions(nc, psA0, ebpool, rparts[11],
                                         cacc11, zb1, t, grain=1024)

        # ---------------- E12 + E22 sims ----------------
        with tc.tile_pool(name="simpsum", bufs=2, space="PSUM") as simpsum:
            if do_sims:
                for t in range(NT):
                    for g in range(L // GRAIN):
                        _e12_tile(nc, simpsum, ebpool, rparts[12], caccX,
                                  zb1, zb2, t, g)
                    _sym_regions(nc, simpsum, ebpool, rparts[22], caccX,
                                 zb2, t)
            else:
                for m in (11, 12, 22):
                    nc.vector.memset(rparts[m][:, :], 1.0)

        # final reductions + packed store
        with tc.tile_pool(name="outpool", bufs=1) as outpool:
            rfin = {}
            for mat in (11, 12, 22):
                nsub = 2 if mat == 12 else 4
                rfin[mat] = outpool.tile([128, NT], F32, name=f"rf{mat}")
                nc.vector.tensor_reduce(
                    out=rfin[mat][:, :],
                    in_=rparts[mat][:, :].rearrange("p (t h) -> p t h",
                                                    h=nsub),
                    axis=mybir.AxisListType.X, op=ALU.add)
            rA = outpool.tile([128, NT], F32, name="rA")
            nc.vector.tensor_tensor(out=rA[:, :], in0=rfin[11][:, :],
                                    in1=rfin[12][:, :], op=ALU.add)
            _dma(nc, outp[0:2048].rearrange("(p t) -> p t", t=NT), rA[:, :])
            _dma(nc, outp[2048:4096].rearrange("(p t) -> p t", t=NT),
                 rfin[22][:, :])
            # column sums of caccX (over partitions); cacc11's went out early
            with tc.tile_pool(name="cspsum", bufs=4, space="PSUM") as cspsum:
                stX = outpool.tile([1, L], F32, name="cstX")
                for c in range(NMC):
                    cs = slice(c * 512, (c + 1) * 512)
                    ps = cspsum.tile([1, 512], F32, name="csps", tag="csps")
                    nc.tensor.matmul(ps[:, :], lhsT=ones_bf[:, :],
                                     rhs=caccX[:, cs], start=True, stop=True)
                    if c % 2 == 0:
                        nc.vector.tensor_copy(stX[0:1, cs], ps[:, :])
                    else:
                        nc.scalar.copy(stX[0:1, cs], ps[:, :])
                _dma(nc, outp[4096 + L:4096 + 2 * L]
                     .rearrange("(o l) -> o l", o=1), stX[:, :])
            # s12 scalar
            st16 = outpool.tile([1, 16], F32, name="st16")
            nc.vector.memset(st16, 0.0)
            nc.vector.tensor_reduce(out=st16[0:1, 0:1], in_=s12parts[:, :],
                                    axis=mybir.AxisListType.X, op=ALU.add)
            _dma(nc, outp[4096 + 2 * L:OUT_N].rearrange("(o l) -> o l", o=1),
                 st16[:, :])
            if fence:
                sinks = [st16, rA, rfin[22], stX] + extra_sinks
                for i, s in enumerate(sinks):
                    nc.vector.tensor_copy(ftile[0:1, i:i + 1], s[0:1, 0:1])


_NC_CACHE = None


def _get_nc():
    global _NC_CACHE
    if _NC_CACHE is None:
        _NC_CACHE = _build_bass()
    return _NC_CACHE


class _Runner:
    """jit-once SPMD runner (mirrors bass2jax.run_bass_via_pjrt multi-core)."""

    def __init__(self, nc):
        import jax
        import jax.numpy as jnp
        from jax.sharding import Mesh, PartitionSpec, NamedSharding
        from jax.experimental.shard_map import shard_map
        from concourse import bass2jax
        import concourse.mybir as _mybir

        bass2jax.install_neuronx_cc_hook()
        self.jax = jax
        in_names, out_names, out_avals = [], [], []
        partition_name = (nc.partition_id_tensor.name
                          if nc.partition_id_tensor else None)
        for alloc in nc.m.functions[0].allocations:
            if not isinstance(alloc, _mybir.MemoryLocationSet):
                continue
            name = alloc.memorylocations[0].name
            if alloc.kind == "ExternalInput":
                if name != partition_name:
                    in_names.append(name)
            elif alloc.kind == "ExternalOutput":
                out_names.append(name)
                out_avals.append(jax.core.ShapedArray(
                    tuple(alloc.tensor_shape), _mybir.dt.np(alloc.dtype)))
        self.in_names, self.out_names, self.out_avals = (
            in_names, out_names, out_avals)
        n_params, n_outs = len(in_names), len(out_names)
        all_names = in_names + out_names
        if partition_name is not None:
            all_names.append(partition_name)

        def _body(*args):
            operands = list(args)
            if partition_name is not None:
                operands.append(bass2jax.partition_id_tensor())
            return tuple(bass2jax._bass_exec_p.bind(
                *operands, out_avals=tuple(out_avals),
                in_names=tuple(all_names), out_names=tuple(out_names),
                lowering_input_output_aliases=(),
                sim_require_finite=True, sim_require_nnan=True, nc=nc))

        devices = jax.devices()[:NCORES]
        self.mesh = Mesh(np.asarray(devices), ("core",))
        self.spec = PartitionSpec("core")
        in_specs = (self.spec,) * (n_params + n_outs)
        out_specs = (self.spec,) * n_outs
        self.fn = jax.jit(shard_map(_body, mesh=self.mesh, in_specs=in_specs,
                                    out_specs=out_specs, check_rep=False),
                          keep_unused=True)
        self.n_params, self.n_outs = n_params, n_outs
        sh = NamedSharding(self.mesh, self.spec)
        shapes = [(NCORES * a.shape[0], *a.shape[1:]) for a in out_avals]
        self._zeros_fn = jax.jit(
            lambda: tuple(jnp.zeros(s, a.dtype)
                          for s, a in zip(shapes, out_avals)),
            out_shardings=tuple(sh for _ in out_avals))

    def put_inputs(self, in_maps):
        import jax
        from jax.sharding import NamedSharding
        sh = NamedSharding(self.mesh, self.spec)
        concat = [np.concatenate([np.asarray(m[n]) for m in in_maps], axis=0)
                  for n in self.in_names]
        return [jax.device_put(a, sh) for a in concat]

    def make_zeros(self):
        return list(self._zeros_fn())

    def run(self, dev_inputs, dev_zeros):
        outs = self.fn(*dev_inputs, *dev_zeros)
        self.jax.block_until_ready(outs)
        return outs

    def results(self, outs):
        res = []
        for c in range(NCORES):
            res.append({
                n: np.asarray(outs[i]).reshape(
                    NCORES, *self.out_avals[i].shape)[c]
                for i, n in enumerate(self.out_names)})
        return res


_RUNNER = None


def _get_runner():
    global _RUNNER
    if _RUNNER is None:
        _RUNNER = _Runner(_get_nc())
    return _RUNNER


def _make_in_maps(X1, X2, W1, b1, W2, b2):
    import ml_dtypes
    bf = ml_dtypes.bfloat16
    wzv = np.stack([
        np.ascontiguousarray(W1.T).reshape(2, 128, D),
        np.ascontiguousarray(W2.T).reshape(2, 128, D),
    ]).astype(bf)
    # the +1 folded into h flows through the bf16 matmul, so subtract the
    # bf16-rounded row sums to match it exactly
    b2p = b2 - W2.astype(bf).astype(np.float64).sum(axis=1)
    bzv = np.stack([b1.reshape(2, 128), b2p.reshape(2, 128)]
                   ).astype(np.float32)
    in_maps = []
    for c in range(NCORES):
        b, s = divmod(c, 2)
        x1 = np.roll(np.ascontiguousarray(X1[b].T), -s * SHARD, axis=1)
        x2 = np.roll(np.ascontiguousarray(X2[b].T), -s * SHARD, axis=1)
        xinv = np.stack([x1.reshape(2, 128, L), x2.reshape(2, 128, L)]
                        ).astype(bf)
        in_maps.append({"xin": xinv, "wz": wzv, "bz": bzv})
    return in_maps


def _finish_host(results):
    """Combine per-core partials into the final scalar loss (float64)."""
    e2 = np.exp(2.0)
    total = 0.0
    for b in range(B):
        o0 = results[2 * b]["outp"].astype(np.float64)
        o1 = results[2 * b + 1]["outp"].astype(np.float64)

        def rpart(o, lo):
            return o[lo:lo + 2048].reshape(128, NT).T.reshape(-1)

        den1 = np.concatenate([
            rpart(o0, 0) + o0[4096:4096 + 2048] + o1[4096 + 2048:4096 + L],
            rpart(o1, 0) + o1[4096:4096 + 2048] + o0[4096 + 2048:4096 + L],
        ]) - e2
        cX0, cX1 = o0[4096 + L:4096 + 2 * L], o1[4096 + L:4096 + 2 * L]
        den2 = np.concatenate([
            rpart(o0, 2048) + cX0[0:2048] + cX1[2048:L],
            rpart(o1, 2048) + cX1[0:2048] + cX0[2048:L],
        ]) - e2
        s12t = o0[4096 + 2 * L] + o1[4096 + 2 * L]
        l1 = np.sum(np.log(den1)) - 2.0 * s12t
        l2 = np.sum(np.log(den2)) - 2.0 * s12t
        total += 0.5 * (l1 + l2)
    return np.float32(total / B)


def kernel(X1, X2, W1, b1, W2, b2):
    X1 = np.asarray(X1, dtype=np.float32)
    X2 = np.asarray(X2, dtype=np.float32)
    W1 = np.asarray(W1, dtype=np.float32)
    b1 = np.asarray(b1, dtype=np.float32)
    W2 = np.asarray(W2, dtype=np.float32)
    b2 = np.asarray(b2, dtype=np.float32)
    r = _get_runner()
    in_maps = _make_in_maps(X1, X2, W1, b1, W2, b2)
    outs = r.run(r.put_inputs(in_maps), r.make_zeros())
    return _finish_host(r.results(outs))
